# revision 1
# baseline (speedup 1.0000x reference)
"""Trainium2 Bass kernel for nn_FullAttentionBlock (B=4, N=1024, D=1024, H=16).

Sharding: 8 cores; core c handles batch c//2, query-row half c%2 (512 rows).
Each core: LN1 + QKV for its whole batch (K/V need all rows), full attention
for all 16 heads over its 512 query rows, out-proj + residual, LN2 + SwiGLU
MLP for its rows. No collectives.

Host prep (weights only + rope/spatial tables):
 - LN/RMS weights and LayerScale gains folded into matmul weights.
 - RoPE pair permutation: within each head, dim order (0,2,..62,1,3,..63) so
   the rotation halves are contiguous; q.k dot products are invariant.
 - Spatial bias collapses to rank-3: bias_h = coords @ A_h @ coords^T with
   A_h = sp_scale * sq_w_h^T @ sk_w_h (3x3).  q/k are augmented with 3 extra
   channels (padded to 128) so scores are one 128-contraction matmul per tile.
 - Softmax denominator via an extra all-ones column in V.
"""

import os
from contextlib import ExitStack

import numpy as np
import ml_dtypes

import concourse.bass as bass
import concourse.tile as tile
from concourse import bacc, mybir
from concourse.bass_utils import run_bass_kernel_spmd

B, N, D, H = 4, 1024, 1024, 16
HD = 64
HID = 4 * D
EPS = 1e-6
THETA = 10000.0
SP_SCALE = 1.0
P = 128
NCORES = 8
MY = N // 2  # 512 query rows per core

bf16 = ml_dtypes.bfloat16
BF = mybir.dt.bfloat16
F32 = mybir.dt.float32
AF = mybir.ActivationFunctionType
OP = mybir.AluOpType


def _f32(x):
    return np.ascontiguousarray(np.asarray(x, np.float32))


def _bf(x):
    return np.ascontiguousarray(np.asarray(x, np.float32).astype(bf16))


# ---------------------------------------------------------------------------
# device program
# ---------------------------------------------------------------------------

def build_program(with_b1=False, with_bo1=False, with_bo2=False):
    nc = bacc.Bacc(
        "TRN2",
        target_bir_lowering=False,
        debug=False,
        enable_asserts=False,
        num_devices=NCORES,
    )

    # --- dram I/O ---
    x_d = nc.dram_tensor("x", (N, D), F32, kind="ExternalInput").ap()
    xbf_d = nc.dram_tensor("x_bf", (N, D), BF, kind="ExternalInput").ap()
    csum_d = nc.dram_tensor("csum_rep", (P, 3 * D), BF, kind="ExternalInput").ap()
    coords_d = nc.dram_tensor("coords_tm", (N, 3), BF, kind="ExternalInput").ap()
    coordsT_d = nc.dram_tensor("coordsT", (3, N), BF, kind="ExternalInput").ap()
    acat_d = nc.dram_tensor("a_cat", (3, H * 3), BF, kind="ExternalInput").ap()
    tq_d = nc.dram_tensor("tq", (MY, 2, 64), BF, kind="ExternalInput").ap()
    tk_d = nc.dram_tensor("tk", (N, 2, 64), BF, kind="ExternalInput").ap()
    wqT_d = nc.dram_tensor("wqT", (D, D), BF, kind="ExternalInput").ap()
    wkT_d = nc.dram_tensor("wkT", (D, D), BF, kind="ExternalInput").ap()
    wvT_d = nc.dram_tensor("wvT", (D, D), BF, kind="ExternalInput").ap()
    owT_d = nc.dram_tensor("owT", (D, D), BF, kind="ExternalInput").ap()
    w2T_d = nc.dram_tensor("w2T", (D, 32, 256), BF, kind="ExternalInput").ap()
    b2_d = nc.dram_tensor("b2", (P, 2 * HID // P), F32, kind="ExternalInput").ap()
    woT_d = nc.dram_tensor("woT", (HID, D), BF, kind="ExternalInput").ap()
    out_d = nc.dram_tensor("out", (MY, D), F32, kind="ExternalOutput").ap()
    if with_b1:
        b1r_d = nc.dram_tensor("b1rep", (P, 3 * D), F32, kind="ExternalInput").ap()
    if with_bo1:
        bo1r_d = nc.dram_tensor("bo1rep", (P, D), F32, kind="ExternalInput").ap()
    if with_bo2:
        bo2r_d = nc.dram_tensor("bo2rep", (P, D), F32, kind="ExternalInput").ap()

    # --- dram scratch ---
    qaug_d = nc.dram_tensor("qaug_s", (MY, H * P), BF, kind="Internal").ap()
    kaug_d = nc.dram_tensor("kaug_s", (N, H * P), BF, kind="Internal").ap()
    xh2_d = nc.dram_tensor("xh2_s", (MY, D), BF, kind="Internal").ap()

    markers = {}
    build_program.markers = markers

    def mark(tag):
        markers[tag] = nc.next_id()

    with tile.TileContext(nc) as tc, ExitStack() as ctx:
        # ---- persistent pools (whole kernel) ----
        pers = ctx.enter_context(tc.tile_pool(name="pers", bufs=1))
        x_res = pers.tile([P, 4, D], F32)        # my x rows (residual)
        vtil = pers.tile([P, 8, H, 65], BF)      # v (token-major) + ones col
        oT_all = pers.tile([P, 8, MY], BF)       # attention out^T, head-major
        x1_sb = pers.tile([P, 4, D], F32)        # x + ls1*o

        small = ctx.enter_context(tc.tile_pool(name="small", bufs=1))
        coordsT_sb = small.tile([3, N], BF)
        acat_sb = small.tile([3, H * 3], BF)
        coords_sb = small.tile([P, 8, 3], BF)
        cq_sb = small.tile([P, 4, H * 3], BF)
        b2_sb = small.tile([P, 2 * HID // P], F32)
        eps_sb = small.tile([P, 1], F32)
        nc.vector.memset(eps_sb[:], EPS)
        nc.sync.dma_start(coordsT_sb[:], coordsT_d)
        nc.sync.dma_start(acat_sb[:], acat_d)
        nc.sync.dma_start(coords_sb[:], coords_d.rearrange("(o p) c -> p o c", p=P))
        nc.sync.dma_start(b2_sb[:], b2_d)
        if with_b1:
            b1r_sb = small.tile([P, 3 * D], F32)
            nc.sync.dma_start(b1r_sb[:], b1r_d)
        if with_bo1:
            bo1r_sb = small.tile([P, D], F32)
            nc.sync.dma_start(bo1r_sb[:], bo1r_d)
        if with_bo2:
            bo2r_sb = small.tile([P, D], F32)
            nc.sync.dma_start(bo2r_sb[:], bo2r_d)

        ln_pool = ctx.enter_context(tc.tile_pool(name="ln", bufs=2))
        st_pool = ctx.enter_context(tc.tile_pool(name="stats", bufs=4))

        def layernorm_chunk(x_ap, out_ap, inv_ap=None, nmi_ap=None):
            """out (bf16) = (x - mean)/sqrt(var + eps); x_ap [128, D] f32."""
            ssum = st_pool.tile([P, 1], F32, tag="ssum")
            nmean = st_pool.tile([P, 1], F32, tag="nmean")
            sq = ln_pool.tile([P, D], BF, tag="lnsq")
            ss = st_pool.tile([P, 1], F32, tag="ss")
            var = st_pool.tile([P, 1], F32, tag="var")
            mu2 = st_pool.tile([P, 1], F32, tag="mu2")
            sd = st_pool.tile([P, 1], F32, tag="sd")
            if inv_ap is None:
                inv_t = st_pool.tile([P, 1], F32, tag="inv")
                inv_ap = inv_t[:]
            if nmi_ap is None:
                nmi_t = st_pool.tile([P, 1], F32, tag="nmi")
                nmi_ap = nmi_t[:]
            inv = inv_ap
            nmi = nmi_ap
            nc.vector.reduce_sum(ssum[:], x_ap, axis=mybir.AxisListType.X)
            nc.vector.tensor_scalar_mul(nmean[:], ssum[:], -1.0 / D)
            nc.scalar.activation(sq[:], x_ap, AF.Square, accum_out=ss[:])
            nc.vector.tensor_tensor(mu2[:], nmean[:], nmean[:], OP.mult)
            nc.vector.scalar_tensor_tensor(
                var[:], ss[:], 1.0 / D, mu2[:], op0=OP.mult, op1=OP.subtract
            )
            nc.scalar.activation(sd[:], var[:], AF.Sqrt, bias=eps_sb[:])
            nc.vector.reciprocal(inv, sd[:])
            nc.vector.tensor_tensor(nmi, nmean[:], inv, OP.mult)
            if out_ap is None:
                return inv, nmi
            nc.scalar.activation(out_ap, x_ap, AF.Identity, bias=nmi, scale=inv)

        # ====== phases A+B+C: LN1-folded qkv, q/k processing ================
        # qkv = inv[t] * (x_bf @ W'^T) - mu[t]*inv[t] * colsum[j]  (LN1 folded
        # into a per-token scalar correction, so the matmul consumes raw x^T
        # transposed straight from the DRAM input with no LN dependency).
        with tc.tile_pool(name="phc", bufs=1) as phc, \
             tc.tile_pool(name="xa", bufs=1) as xa_pool, \
             tc.tile_pool(name="qkproc", bufs=2) as qk_pool, \
             tc.tile_pool(name="cqpsum", bufs=2, space="PSUM") as cq_psum, \
             tc.tile_pool(name="qkvpsum", bufs=3, space="PSUM") as psum:
            xT = phc.tile([P, 8, N], BF)             # raw x^T (bf16)
            wq_sb = phc.tile([P, 8, D], BF)
            wk_sb = phc.tile([P, 8, D], BF)
            wv_sb = phc.tile([P, 8, D], BF)
            wk_r = wkT_d.rearrange("(o p) f -> p o f", p=P)
            # one copy then the transposes: alternating copy/transpose pays
            # the xbar-mode serialization on every switch
            nc.sync.dma_start(wk_sb[:], wk_r)
            nc.sync.dma_start_transpose(xT[:, :, 0:P], xbf_d[0:P, :])
            nc.sync.dma_start_transpose(xT[:, :, P:2 * P], xbf_d[P:2 * P, :])
            csum_sb = phc.tile([P, 3 * D], BF)
            xtm = xa_pool.tile([P, 8, D], BF, name="xtm")
            xtm_r = xbf_d.rearrange("(o p) f -> p o f", p=P)
            nc.sync.dma_start(xtm[:, 0:2], xtm_r[:, 0:2])
            nc.sync.dma_start(csum_sb[:, D:2 * D], csum_d[:, D:2 * D])
            tqtk = phc.tile([P, 12, 2, 64], BF)      # rope tables: q 0..3, k 4..11
            nc.sync.dma_start(tqtk[:, 4:12], tk_d.rearrange("(o p) a b -> p o a b", p=P))
            wq_r = wqT_d.rearrange("(o p) f -> p o f", p=P)
            nc.sync.dma_start(wq_sb[:, 0:4], wq_r[:, 0:4])
            nc.sync.dma_start(xtm[:, 2:4], xtm_r[:, 2:4])
            nc.sync.dma_start(wq_sb[:, 4:8], wq_r[:, 4:8])
            for t8 in range(2, 8):
                nc.sync.dma_start_transpose(
                    xT[:, :, t8 * P:(t8 + 1) * P],
                    xbf_d[t8 * P:(t8 + 1) * P, :],
                )
            nc.sync.dma_start(csum_sb[:, 0:D], csum_d[:, 0:D])
            nc.sync.dma_start(tqtk[:, 0:4], tq_d.rearrange("(o p) a b -> p o a b", p=P))
            nc.sync.dma_start(wv_sb[:], wvT_d.rearrange("(o p) f -> p o f", p=P))
            nc.sync.dma_start(csum_sb[:, 2 * D:], csum_d[:, 2 * D:])
            # ping-pong aug buffers: pad cols 67..128 stay zero across reuse
            aug_bufs = [phc.tile([P, H, P], BF, name=f"augbuf{i}")
                        for i in range(2)]
            nc.vector.memset(aug_bufs[0][:], 0.0)
            nc.vector.memset(aug_bufs[1][:], 0.0)
            nc.vector.memset(vtil[:, :, :, 64:65], 1.0)

            nc.sync.dma_start(xtm[:, 4:8], xtm_r[:, 4:8])
            inv8 = phc.tile([P, 8], F32)
            nmi8 = phc.tile([P, 8], F32)
            # batched LN1 stats, two groups of 4 chunks
            ssum8 = phc.tile([P, 8], F32)
            ss8 = phc.tile([P, 8], F32)
            sq_s = phc.tile([P, D], BF)
            for grp in range(4):
                for i in range(2):
                    t8 = grp * 2 + i
                    x_ap = xtm[:, t8, :]
                    nc.vector.reduce_sum(ssum8[:, t8:t8 + 1], x_ap,
                                         axis=mybir.AxisListType.X)
                    nc.scalar.activation(sq_s[:], x_ap, AF.Square,
                                         accum_out=ss8[:, t8:t8 + 1])
                g = slice(grp * 2, grp * 2 + 2)
                nmean = st_pool.tile([P, 2], F32, tag="b_nmean")
                mu2 = st_pool.tile([P, 2], F32, tag="b_mu2")
                var = st_pool.tile([P, 2], F32, tag="b_var")
                sd = st_pool.tile([P, 2], F32, tag="b_sd")
                nc.vector.tensor_scalar_mul(nmean[:], ssum8[:, g], -1.0 / D)
                nc.vector.tensor_tensor(mu2[:], nmean[:], nmean[:], OP.mult)
                nc.vector.scalar_tensor_tensor(
                    var[:], ss8[:, g], 1.0 / D, mu2[:],
                    op0=OP.mult, op1=OP.subtract)
                nc.scalar.activation(sd[:], var[:], AF.Sqrt, bias=eps_sb[:])
                nc.vector.reciprocal(inv8[:, g], sd[:])
                nc.vector.tensor_tensor(nmi8[:, g], nmean[:], inv8[:, g],
                                        OP.mult)
            stats = [(inv8[:, t:t + 1], nmi8[:, t:t + 1]) for t in range(8)]

            # cq = coords @ A_cat  (my 4 chunks) -> sbuf
            for t4 in range(4):
                cp = cq_psum.tile([P, H * 3], F32, tag="cqp")
                nc.tensor.matmul(
                    cp[:], coordsT_sb[:, t4 * P:(t4 + 1) * P], acat_sb[:],
                    start=True, stop=True,
                )
                nc.scalar.copy(cq_sb[:, t4, :], cp[:])

            def proj_corrected(w_sb, t8, which, bias_off=None, out_ap=None):
                """ln-corrected token-major projection [128, D] bf16 sbuf."""
                ps = psum.tile([P, D], F32, tag="qkv_ps")
                for hf in range(2):
                    for dc in range(8):
                        nc.tensor.matmul(
                            ps[:, hf * 512:(hf + 1) * 512],
                            xT[:, dc, t8 * P:(t8 + 1) * P],
                            w_sb[:, dc, hf * 512:(hf + 1) * 512],
                            start=(dc == 0),
                            stop=(dc == 7),
                        )
                inv, nmi = stats[t8]
                ta = qk_pool.tile([P, D], BF, tag="ta")
                nc.scalar.activation(ta[:], ps[:], AF.Identity, scale=inv)
                co = which * D
                if out_ap is None:
                    dst = qk_pool.tile([P, D], BF, tag="corr")
                    out_ap = dst[:]
                    csrc = csum_sb[:, co:co + D]
                    tsrc = ta[:]
                else:
                    csrc = csum_sb[:, co:co + D].rearrange(
                        "p (h d) -> p h d", h=H)
                    tsrc = ta[:].rearrange("p (h d) -> p h d", h=H)
                nc.vector.scalar_tensor_tensor(
                    out_ap, csrc, nmi, tsrc, op0=OP.mult, op1=OP.add,
                )
                if bias_off is not None:
                    nc.vector.tensor_tensor(
                        out_ap, out_ap, b1r_sb[:, bias_off:bias_off + D], OP.add
                    )
                return out_ap

            def qk_process(src_ap, tbl_i, t8, aug_fn, dst_dram):
                """rms-norm + rope on token-major q/k chunk; writes aug tile."""
                sqs = qk_pool.tile([P, D], BF, tag="sqs")
                nc.scalar.activation(sqs[:], src_ap, AF.Square)
                ss = st_pool.tile([P, H], F32, tag="rms_ss")
                nc.vector.reduce_sum(
                    ss[:], sqs[:].rearrange("p (h d) -> p h d", h=H),
                    axis=mybir.AxisListType.X,
                )
                sd = st_pool.tile([P, H], F32, tag="rms_sd")
                nc.scalar.activation(sd[:], ss[:], AF.Sqrt, scale=1.0 / HD,
                                     bias=eps_sb[:])
                rs = st_pool.tile([P, H], F32, tag="rms_rs")
                nc.vector.reciprocal(rs[:], sd[:])
                qs = qk_pool.tile([P, H, HD], BF, tag="qs")
                nc.vector.tensor_tensor(
                    qs[:], src_ap.rearrange("p (h d) -> p h d", h=H),
                    rs[:, :, None].to_broadcast((P, H, HD)), OP.mult,
                )
                aug = aug_bufs[qk_process.flip][:]
                qk_process.flip ^= 1
                # rope via 2 ops: prods[p,h,j,d] = qs[p,h,d] * tb[p,j,d]
                # (tb[0] = [cos*w1 | -sin*w2], tb[1] = [sin*w1 | cos*w2]),
                # then y[j] = prods[j,0:32] + prods[j,32:64].
                tb = tqtk[:, tbl_i, None, :, :].to_broadcast((P, 1, 2, HD))
                prods = qk_pool.tile([P, H, 2, HD], BF, tag="prods")
                nc.vector.tensor_tensor(
                    prods[:], qs[:, :, None, :].to_broadcast((P, H, 2, HD)),
                    tb.to_broadcast((P, H, 2, HD)), OP.mult,
                )
                nc.vector.tensor_tensor(
                    aug[:, :, 0:64].rearrange("p h (j d) -> p h j d", j=2),
                    prods[:, :, :, 0:32], prods[:, :, :, 32:64], OP.add,
                )
                aug_fn(aug)
                return nc.sync.dma_start(
                    dst_dram[t8 * P:(t8 + 1) * P, :],
                    aug[:].rearrange("p h d -> p (h d)"),
                )

            mark('qkv_loop')
            qk_process.flip = 0

            k_store = {}

            def do_k(t8):
                k_src = proj_corrected(wk_sb, t8, 1,
                                       bias_off=D if with_b1 else None)

                def k_aug(aug, t8=t8):
                    nc.vector.tensor_copy(
                        out=aug[:, :, 64:67],
                        in_=coords_sb[:, t8:t8 + 1, :].to_broadcast((P, H, 3)),
                    )

                k_store[t8] = qk_process(k_src, 4 + t8, t8, k_aug, kaug_d)

            def do_q(t4):
                q_src = proj_corrected(wq_sb, t4, 0,
                                       bias_off=0 if with_b1 else None)

                def q_aug(aug):
                    nc.vector.tensor_copy(
                        out=aug[:, :, 64:67],
                        in_=cq_sb[:, t4, :].rearrange("p (h c) -> p h c", h=H),
                    )

                qk_process(q_src, t4, t4, q_aug, qaug_d)

            def do_v(t8):
                proj_corrected(wv_sb, t8, 2,
                               bias_off=2 * D if with_b1 else None,
                               out_ap=vtil[:, t8, :, 0:64])

            # k and q early (attention transposes depend on them), v late:
            # its short evac chain lets attention score matmuls interleave
            # with the v-projection tail on PE.
            do_k(0)
            do_k(1)
            for t8 in range(4):
                do_q(t8)
                do_k(t8 + 2)
            do_k(6)
            do_v(0)
            do_k(7)
            do_v(1)
            for t8 in range(2, 8):
                do_v(t8)

        mark('phaseD')
        # ==================== phase D: attention per head ===================
        ow_ctx = ExitStack()
        ow_pool = ow_ctx.enter_context(tc.tile_pool(name="ow", bufs=1))
        owT_sb = ow_pool.tile([P, 8, D], BF)
        ow_r = owT_d.rearrange("(o p) f -> p o f", p=P)
        ow_inst = nc.sync.dma_start(owT_sb[:], ow_r)
        # pin the 2MB prefetch into mid-phase-C DMA slack so its transfer
        # cannot starve the latency-critical head transposes at the boundary
        tile.add_dep_helper(ow_inst.ins, k_store[2].ins,
                            reason="owT prefetch after mid-C")
        with tc.tile_pool(name="att", bufs=2) as att_pool, \
             tc.tile_pool(name="apsum", bufs=3, space="PSUM") as apsum, \
             tc.tile_pool(name="opsum", bufs=2, space="PSUM") as opsum:
            kT2 = qT2 = None
            xr_r = x_d[0:MY].rearrange("(o p) f -> p o f", p=P)
            for h in range(H):
                if h in (9, 11, 13, 15):
                    # residual fp32 rows (needed at phase E); spread between
                    # late heads so they never block the head transposes
                    i = (h - 9) // 2
                    nc.scalar.dma_start(x_res[:, i:i + 1], xr_r[:, i:i + 1])

                if h % 2 == 0:
                    kT2 = att_pool.tile([P, 2, N], BF, tag="kT")
                    nc.sync.dma_start_transpose(
                        kT2[:], kaug_d[:, h * P:(h + 2) * P])
                    qT2 = att_pool.tile([P, 2, MY], BF, tag="qT")
                    nc.sync.dma_start_transpose(
                        qT2[:], qaug_d[:, h * P:(h + 2) * P])
                kT = kT2[:, h % 2]
                qT = qT2[:, h % 2]
                expT = att_pool.tile([P, 8, MY], BF, tag="expT")
                for kc2 in range(4):
                    s_ps = apsum.tile([P, 2, MY], F32, tag="s_ps")
                    for j in range(2):
                        nc.tensor.matmul(
                            s_ps[:, j],
                            kT[:, (2 * kc2 + j) * P:(2 * kc2 + j + 1) * P], qT,
                            start=True, stop=True,
                        )
                    nc.scalar.activation(
                        expT[:, 2 * kc2:2 * kc2 + 2, :], s_ps[:],
                        AF.Exp, scale=0.125
                    )
                o_ps = opsum.tile([65, MY], F32, tag="o_ps")
                for kc in range(8):
                    nc.tensor.matmul(
                        o_ps[:], vtil[:, kc, h, :], expT[:, kc, :],
                        start=(kc == 0), stop=(kc == 7),
                    )
                rec = att_pool.tile([1, MY], F32, tag="rec")
                nc.vector.reciprocal(rec[:], o_ps[64:65, :])
                bc = att_pool.tile([64, MY], F32, tag="bc")
                nc.gpsimd.partition_broadcast(bc[:], rec[:])
                nc.vector.tensor_tensor(
                    oT_all[(h % 2) * 64:(h % 2) * 64 + 64, h // 2, :],
                    o_ps[0:64, :], bc[:], OP.mult,
                )

        mark('phaseE')
        # ================== phase E: out-proj + residual ====================
        with tc.tile_pool(name="ebuf", bufs=2) as ebuf, \
             tc.tile_pool(name="epsum", bufs=4, space="PSUM") as epsum:
            for qc in range(4):
                for eh in range(2):
                    xp = epsum.tile([P, 512], F32, tag="xp")
                    for jc in range(8):
                        nc.tensor.matmul(
                            xp[:],
                            oT_all[:, jc, qc * P:(qc + 1) * P],
                            owT_sb[:, jc, eh * 512:(eh + 1) * 512],
                            start=(jc == 0), stop=(jc == 7),
                        )
                    sl = slice(eh * 512, (eh + 1) * 512)
                    if with_bo1:
                        xb = ebuf.tile([P, 512], F32, tag="xpb")
                        nc.vector.tensor_tensor(xb[:], xp[:], bo1r_sb[:, sl],
                                                OP.add)
                        nc.vector.tensor_tensor(
                            x1_sb[:, qc, sl], x_res[:, qc, sl], xb[:], OP.add
                        )
                    else:
                        nc.vector.tensor_tensor(
                            x1_sb[:, qc, sl], x_res[:, qc, sl], xp[:], OP.add
                        )

        ow_ctx.close()
        mark('phaseF')
        # =================== phases F+G: LN2 + MLP up =======================
        with tc.tile_pool(name="mlp_pers", bufs=1) as mlp_pers:
            xh2T = mlp_pers.tile([P, 8, MY], BF)
            actT = mlp_pers.tile([P, 32, MY], BF)
            for qc in range(4):
                xh2_t = ln_pool.tile([P, D], BF, tag="xh2")
                layernorm_chunk(x1_sb[:, qc, :], xh2_t[:])
                nc.sync.dma_start(xh2_d[qc * P:(qc + 1) * P, :], xh2_t[:])
                nc.sync.dma_start_transpose(
                    xh2T[:, :, qc * P:(qc + 1) * P],
                    xh2_d[qc * P:(qc + 1) * P, :],
                )

            with tc.tile_pool(name="w2", bufs=3) as w2_pool, \
                 tc.tile_pool(name="sil", bufs=2) as sil_pool, \
                 tc.tile_pool(name="gpsum", bufs=4, space="PSUM") as gpsum:
                for jj in range(32):
                    w2_t = w2_pool.tile([P, 8, 256], BF, tag="w2t")
                    nc.sync.dma_start(
                        w2_t[:],
                        w2T_d[:, jj, :].rearrange("(o p) f -> p o f", p=P),
                    )
                    ups = []
                    for half in range(2):
                        up = gpsum.tile([P, MY], F32, tag="u_ps")
                        if jj < 4:
                            # token-split groups: the first half only needs
                            # token chunks 0-1 of xh2T, so these matmuls can
                            # start while LN2 of chunks 2-3 is still going
                            for th in range(2):
                                tsl = slice(th * 256, (th + 1) * 256)
                                for dc in range(8):
                                    nc.tensor.matmul(
                                        up[:, tsl],
                                        w2_t[:, dc, half * P:(half + 1) * P],
                                        xh2T[:, dc, tsl],
                                        start=(dc == 0), stop=(dc == 7),
                                    )
                        else:
                            for dc in range(8):
                                nc.tensor.matmul(
                                    up[:],
                                    w2_t[:, dc, half * P:(half + 1) * P],
                                    xh2T[:, dc, :],
                                    start=(dc == 0), stop=(dc == 7),
                                )
                        ups.append(up)
                    sil = sil_pool.tile([P, MY], F32, tag="sil")
                    nc.scalar.activation(
                        sil[:], ups[0][:], AF.Silu, bias=b2_sb[:, jj:jj + 1]
                    )
                    nc.vector.scalar_tensor_tensor(
                        actT[:, jj, :], ups[1][:], b2_sb[:, jj + 32:jj + 33],
                        sil[:], op0=OP.add, op1=OP.mult,
                    )

            mark('phaseH')
            # ============ phase H: MLP down + residual + out ================
            with tc.tile_pool(name="wo", bufs=3) as wo_pool, \
                 tc.tile_pool(name="outp", bufs=3) as out_pool, \
                 tc.tile_pool(name="mpsum", bufs=1, space="PSUM") as mpsum:
                for eh in range(2):
                    sl = slice(eh * 512, (eh + 1) * 512)
                    mps = [mpsum.tile([P, 512], F32, tag=f"m_ps{qc}",
                                      name=f"m_ps{qc}_{eh}")
                           for qc in range(4)]
                    for h4 in range(8):
                        wo_t = wo_pool.tile([P, 4, 512], BF, tag="wot")
                        nc.sync.dma_start(
                            wo_t[:],
                            woT_d[h4 * 512:(h4 + 1) * 512, sl].rearrange(
                                "(o p) f -> p o f", p=P),
                        )
                        for hi in range(4):
                            hc = h4 * 4 + hi
                            for qc in range(4):
                                nc.tensor.matmul(
                                    mps[qc][:],
                                    actT[:, hc, qc * P:(qc + 1) * P],
                                    wo_t[:, hi, :],
                                    start=(hc == 0), stop=(hc == 31),
                                )
                    for qc in range(4):
                        o_t = out_pool.tile([P, 512], F32, tag="outt")
                        if with_bo2:
                            ob = out_pool.tile([P, 512], F32, tag="outb")
                            nc.vector.tensor_tensor(ob[:], mps[qc][:],
                                                    bo2r_sb[:, sl], OP.add)
                            nc.vector.tensor_tensor(o_t[:], x1_sb[:, qc, sl],
                                                    ob[:], OP.add)
                        else:
                            nc.vector.tensor_tensor(
                                o_t[:], x1_sb[:, qc, sl], mps[qc][:], OP.add
                            )
                        nc.scalar.dma_start(out_d[qc * P:(qc + 1) * P, sl], o_t[:])

    mark('end')
    nc.compile()
    return nc


# ---------------------------------------------------------------------------
# host side
# ---------------------------------------------------------------------------

_prog_cache = {}


def _get_program(flags):
    if flags not in _prog_cache:
        _prog_cache[flags] = build_program(*flags)
    return _prog_cache[flags]


def kernel(**inputs):
    x = _f32(inputs["x"])
    coords = _f32(inputs["coords"])
    rope_pos = np.asarray(inputs["rope_pos"])
    ln1_w, ln1_b = _f32(inputs["ln1_w"]), _f32(inputs["ln1_b"])
    qkv_w, qkv_b = _f32(inputs["qkv_w"]), _f32(inputs["qkv_b"])
    qnw, knw = _f32(inputs["q_norm_w"]), _f32(inputs["k_norm_w"])
    sq_w, sk_w = _f32(inputs["sq_w"]), _f32(inputs["sk_w"])
    out_w, out_b = _f32(inputs["out_w"]), _f32(inputs["out_b"])
    ls1 = _f32(inputs["ls1_g"])
    ln2_w, ln2_b = _f32(inputs["ln2_w"]), _f32(inputs["ln2_b"])
    w12_w, w12_b = _f32(inputs["w12_w"]), _f32(inputs["w12_b"])
    wo_w, wo_b = _f32(inputs["wo_w"]), _f32(inputs["wo_b"])
    ls2 = _f32(inputs["ls2_g"])

    # ---- weight folding ----
    W1 = qkv_w * ln1_w[None, :]
    b1 = qkv_w @ ln1_b + qkv_b
    perm = np.empty(HD, np.int64)
    perm[:32] = np.arange(32) * 2
    perm[32:] = np.arange(32) * 2 + 1
    permD = np.concatenate([h * HD + perm for h in range(H)])
    Wq = W1[:D][permD]
    Wk = W1[D:2 * D][permD]
    Wv = W1[2 * D:]
    b1p = np.concatenate([b1[:D][permD], b1[D:2 * D][permD], b1[2 * D:]])
    qnw_p, knw_p = qnw[perm], knw[perm]

    half = 32
    inv_freq = 1.0 / THETA ** (np.arange(half, dtype=np.float32) / half)
    freqs = rope_pos.astype(np.float32)[:, None] * inv_freq
    cos, sin = np.cos(freqs), np.sin(freqs)
    def rope_tbl(w):
        t = np.empty((N, 2, 64), np.float32)
        t[:, 0, :32] = cos * w[None, :32]
        t[:, 0, 32:] = -sin * w[None, 32:]
        t[:, 1, :32] = sin * w[None, :32]
        t[:, 1, 32:] = cos * w[None, 32:]
        return t

    tq = rope_tbl(qnw_p)
    tk = rope_tbl(knw_p)

    A_cat = np.concatenate(
        [SP_SCALE * sq_w[h * HD:(h + 1) * HD].T @ sk_w[h * HD:(h + 1) * HD]
         for h in range(H)], 1)  # (3, 48)

    Wo1 = out_w * ls1[:, None]
    bo1 = ls1 * out_b
    W2 = w12_w * ln2_w[None, :]
    b2 = w12_w @ ln2_b + w12_b
    Wo2 = wo_w * ls2[:, None]
    bo2 = ls2 * wo_b

    with_b1 = bool(np.any(b1p != 0))
    with_bo1 = bool(np.any(bo1 != 0))
    with_bo2 = bool(np.any(bo2 != 0))
    flags = (with_b1, with_bo1, with_bo2)
    nc = _get_program(flags)

    Wqb = _bf(Wq).astype(np.float32)
    Wkb = _bf(Wk).astype(np.float32)
    Wvb = _bf(Wv).astype(np.float32)
    csum = np.concatenate([Wqb.sum(1), Wkb.sum(1), Wvb.sum(1)])  # (3D,)
    w2T = _bf(W2.T)  # (D, 2*HID)
    w2pair = np.empty((D, 32, 256), bf16)
    w2pair[:, :, :128] = w2T[:, :HID].reshape(D, 32, 128)
    w2pair[:, :, 128:] = w2T[:, HID:].reshape(D, 32, 128)
    shared = {
        "a_cat": _bf(A_cat),
        "wqT": _bf(Wq.T), "wkT": _bf(Wk.T), "wvT": _bf(Wv.T),
        "owT": _bf(Wo1.T),
        "w2T": np.ascontiguousarray(w2pair),
        "csum_rep": _bf(np.broadcast_to(csum[None, :], (P, 3 * D))),
        "b2": _f32(b2.reshape(2 * HID // P, P).T),
        "woT": _bf(Wo2.T),
    }
    if with_b1:
        shared["b1rep"] = _f32(np.broadcast_to(b1p[None, :], (P, 3 * D)))
    if with_bo1:
        shared["bo1rep"] = _f32(np.broadcast_to(bo1[None, :], (P, D)))
    if with_bo2:
        shared["bo2rep"] = _f32(np.broadcast_to(bo2[None, :], (P, D)))

    in_maps = []
    for c in range(NCORES):
        b, r = c // 2, c % 2
        rows = np.concatenate([np.arange(r * MY, (r + 1) * MY),
                               np.arange(0, r * MY),
                               np.arange((r + 1) * MY, N)])
        m = dict(shared)
        m["x"] = _f32(x[b][rows])
        m["x_bf"] = _bf(x[b][rows])
        m["coords_tm"] = _bf(coords[b][rows])
        m["coordsT"] = _bf(coords[b][rows].T)
        m["tq"] = _bf(tq[r * MY:(r + 1) * MY])
        m["tk"] = _bf(tk[rows])
        in_maps.append(m)

    res = run_bass_kernel_spmd(nc, in_maps, core_ids=list(range(NCORES)),
                               trace=bool(int(os.environ.get("K_TRACE", "0"))))
    out = np.empty((B, N, D), np.float32)
    for c in range(NCORES):
        b, r = c // 2, c % 2
        out[b, r * MY:(r + 1) * MY] = res.results[c]["out"]
    kernel.last_result = res
    return out



# revision 56
# speedup vs baseline: 1.7588x; 1.7588x over previous
"""Trainium2 Bass kernel for nn_FullAttentionBlock (B=4, N=1024, D=1024, H=16).

Sharding: 8 cores; core c handles batch c//2, query-row half c%2 (512 rows).
Each core: QKV for its whole batch (K/V need all rows), full attention for all
16 heads over its 512 query rows, out-proj + residual, LN2 + SwiGLU MLP.

Perf design (vs bf16 baseline):
 - fp8(e4m3) DoubleRow matmuls for QKV / AV / out-proj / MLP: 2x contraction
   per pass at 0.5 cycles/row.  Weights pre-scaled by powers of 2 into fp8
   range; descale folded into existing evac scales.
 - LayerNorm mean handling folded into mean-centered weight matrices
   (W0 = W - rowmean(W) gives W0@x == W@(x-mu) exactly); only the variance
   is computed on device.
 - LN1 scale for Q/K paths is dropped entirely: rms qk-norm divides it out.
 - Host ships x in fp8 both token-major and pre-transposed (dim-major), so
   no on-device x transposes.
 - Scores matmul stays bf16 (fp8 DoubleRow needs pair-stride >= 16B, which
   the transposed aug layout cannot give without extra converts).
 - Softmax denominator via extra ones-column in V (DoubleRow-friendly 66-wide
   v tiles); exp batched [P,2,512] on Act.
"""

import os
from contextlib import ExitStack

import numpy as np
import ml_dtypes

import concourse.bass as bass
import concourse.tile as tile
from concourse import bacc, mybir
from concourse.bass_utils import run_bass_kernel_spmd

B, N, D, H = 4, 1024, 1024, 16
HD = 64
HID = 4 * D
EPS = 1e-6
THETA = 10000.0
SP_SCALE = 1.0
P = 128
NCORES = 8
MY = N // 2  # 512 query rows per core

SQ = 32.0          # fp8 pre-scale for Wv / W2
SO = float(2 ** 21)  # fp8 pre-scale for ls-folded out/wo weights
SSP = 1.0          # spatial q-side scale (aug is bf16; no range issue)

bf16 = ml_dtypes.bfloat16
f8e4 = ml_dtypes.float8_e4m3fn
BF = mybir.dt.bfloat16
F8 = mybir.dt.float8e4
F32 = mybir.dt.float32
AF = mybir.ActivationFunctionType
OP = mybir.AluOpType
DRM = mybir.MatmulPerfMode.DoubleRow


def _f32(x):
    return np.ascontiguousarray(np.asarray(x, np.float32))


def _bf(x):
    return np.ascontiguousarray(np.asarray(x, np.float32).astype(bf16))


def _q8(x):
    x = np.asarray(x, np.float32)
    return np.ascontiguousarray(np.clip(x, -448.0, 448.0).astype(f8e4))


# ---------------------------------------------------------------------------
# device program
# ---------------------------------------------------------------------------

def build_program(with_b1=False, with_bo1=False, with_bo2=False):
    nc = bacc.Bacc(
        "TRN2",
        target_bir_lowering=False,
        debug=False,
        enable_asserts=False,
        num_devices=NCORES,
    )

    # --- dram inputs ---
    x8T_d = nc.dram_tensor("x8T", (8, P, N), F8, kind="ExternalInput").ap()
    xsqT_d = nc.dram_tensor("xsqT", (8, P, N), F8, kind="ExternalInput").ap()
    xres_d = nc.dram_tensor("xres", (MY, D), F32, kind="ExternalInput").ap()
    coords_d = nc.dram_tensor("coords_tm", (N, 3), BF, kind="ExternalInput").ap()
    coordsT_d = nc.dram_tensor("coordsT", (3, MY), BF, kind="ExternalInput").ap()
    acat_d = nc.dram_tensor("a_cat", (3, H * 3), BF, kind="ExternalInput").ap()
    tq_d = nc.dram_tensor("tq", (MY, 2, 64), BF, kind="ExternalInput").ap()
    tk_d = nc.dram_tensor("tk", (N, 2, 64), BF, kind="ExternalInput").ap()
    wqkv_d = nc.dram_tensor("wqkv", (3, 8, P, D), F8, kind="ExternalInput").ap()
    ow8_d = nc.dram_tensor("ow8", (8, P, D), F8, kind="ExternalInput").ap()
    ow8T_d = nc.dram_tensor("ow8T", (P, 4, 2, D), F8, kind="ExternalInput").ap()
    w28_d = nc.dram_tensor("w28", (32, P, 8, 256), F8, kind="ExternalInput").ap()
    wo8_d = nc.dram_tensor("wo8", (32, P, D), F8, kind="ExternalInput").ap()
    out_d = nc.dram_tensor("out", (MY, D), F32, kind="ExternalOutput").ap()
    if with_b1:
        b1r_d = nc.dram_tensor("b1rep", (P, 3 * D), F32, kind="ExternalInput").ap()
    if with_bo1:
        bo1r_d = nc.dram_tensor("bo1rep", (P, D), F32, kind="ExternalInput").ap()
    if with_bo2:
        bo2r_d = nc.dram_tensor("bo2rep", (P, D), F32, kind="ExternalInput").ap()

    # --- dram scratch ---
    kaug_d = nc.dram_tensor("kaug_s", (N, H, P), BF, kind="Internal").ap()
    qaug_d = nc.dram_tensor("qaug_s", (MY, H, P), BF, kind="Internal").ap()
    inv2_d = nc.dram_tensor("inv2_s", (P, 4), BF, kind="Internal").ap()

    with tile.TileContext(nc) as tc, ExitStack() as ctx:
        # ---- persistent pools ----
        pers = ctx.enter_context(tc.tile_pool(name="pers", bufs=1))
        vtil = pers.tile([P, 8, H, 66], F8)      # v slot-major + ones col
        oT_all = pers.tile([P, 8, MY], F8)       # attn out^T, slot-major
        x1_sb = pers.tile([P, 4, D], F32)

        small = ctx.enter_context(tc.tile_pool(name="small", bufs=1))
        coords_sb = small.tile([P, 8, 3], BF)
        coordsT_sb = small.tile([3, MY], BF)
        acat_sb = small.tile([3, H * 3], BF)
        eps_sb = small.tile([P, 1], F32)
        epsk_sb = small.tile([P, 1], F32)        # 1024*eps for LN sqrt
        inv32 = small.tile([P, 8], F32)          # (1/32)*rsqrt(var+eps)
        ssq8 = small.tile([P, 8], F32)           # LN1 sum(x^2)
        inv232 = small.tile([P, 4], F32)
        nc.vector.memset(eps_sb[:], EPS)
        nc.vector.memset(epsk_sb[:], EPS * 1024.0)
        nc.sync.dma_start(coords_sb[:], coords_d.rearrange("(o p) c -> p o c", p=P))
        nc.sync.dma_start(coordsT_sb[:], coordsT_d)
        nc.sync.dma_start(acat_sb[:], acat_d)
        if with_b1:
            b1r_sb = small.tile([P, 3 * D], F32)
            nc.sync.dma_start(b1r_sb[:], b1r_d)
        if with_bo1:
            bo1r_sb = small.tile([P, D], F32)
            nc.sync.dma_start(bo1r_sb[:], bo1r_d)
        if with_bo2:
            bo2r_sb = small.tile([P, D], F32)
            nc.sync.dma_start(bo2r_sb[:], bo2r_d)

        st_pool = ctx.enter_context(tc.tile_pool(name="stats", bufs=4))
        x8p = ctx.enter_context(tc.tile_pool(name="x8p", bufs=1))
        x8T = x8p.tile([P, 8, N], F8)
        attk_pool = ctx.enter_context(tc.tile_pool(name="attk", bufs=2))
        attq_pool = ctx.enter_context(tc.tile_pool(name="attq", bufs=2))
        kts, qts, tr_insts = [], [], []
        kaug_flat = kaug_d.rearrange("n h d -> n (h d)")
        qaug_flat = qaug_d.rearrange("n h d -> n (h d)")

        def issue_k_group(g):
            kT4 = attk_pool.tile([P, 4, N], BF, tag="kT")
            nc.sync.dma_start_transpose(
                kT4[:, :, 0:MY],
                kaug_flat[0:MY, 4 * g * P:(4 * g + 4) * P])
            nc.sync.dma_start_transpose(
                kT4[:, :, MY:N],
                kaug_flat[MY:N, 4 * g * P:(4 * g + 4) * P])
            kts.append(kT4)

        def issue_q_group(g):
            qT4 = attq_pool.tile([P, 4, MY], BF, tag="qT")
            ti = nc.sync.dma_start_transpose(
                qT4[:], qaug_flat[:, 4 * g * P:(4 * g + 4) * P])
            tr_insts.append(ti)
            qts.append(qT4)

        # ====== phase C: QKV + q/k rope/rms + v ============================
        phc_ctx = ExitStack()
        phc = phc_ctx.enter_context(tc.tile_pool(name="phc", bufs=1))
        qk_pool = phc_ctx.enter_context(tc.tile_pool(name="qkproc", bufs=3))
        cq_psum = phc_ctx.enter_context(
            tc.tile_pool(name="cqpsum", bufs=1, space="PSUM"))
        psum = phc_ctx.enter_context(
            tc.tile_pool(name="qkvpsum", bufs=3, space="PSUM"))

        wq_sb = phc.tile([P, 8, D], F8)
        wk_sb = phc.tile([P, 8, D], F8)
        wv_sb = phc.tile([P, 8, D], F8)
        tqtk = phc.tile([P, 12, 2, 64], BF)
        ones8 = phc.tile([P, 1], F8)
        nc.vector.memset(ones8[:], 1.0)
        nc.vector.memset(vtil[:, :, :, 64:65], 1.0)
        nc.vector.memset(vtil[:, :, :, 65:66], 0.0)

        x8T_r = x8T_d.rearrange("s p t -> p s t")
        nc.sync.dma_start(x8T[:, 0:2], x8T_r[:, 0:2])
        nc.sync.dma_start(wk_sb[:], wqkv_d[1].rearrange("s p f -> p s f"))
        nc.sync.dma_start(x8T[:, 2:4], x8T_r[:, 2:4])
        nc.sync.dma_start(tqtk[:, 4:12], tk_d.rearrange("(o p) a b -> p o a b", p=P))
        nc.sync.dma_start(x8T[:, 4:8], x8T_r[:, 4:8])
        nc.sync.dma_start(wq_sb[:], wqkv_d[0].rearrange("s p f -> p s f"))
        nc.sync.dma_start(tqtk[:, 0:4], tq_d.rearrange("(o p) a b -> p o a b", p=P))
        nc.sync.dma_start(wv_sb[:], wqkv_d[2].rearrange("s p f -> p s f"))

        # ping-pong aug buffers; pad cols stay zero
        aug_bufs = [phc.tile([P, H, P], BF, name=f"augbuf{i}") for i in range(2)]
        nc.vector.memset(aug_bufs[0][:], 0.0)
        nc.vector.memset(aug_bufs[1][:], 0.0)

        sx_ps = cq_psum.tile([P, 24], F32)   # ln1 sums: x (0:8), x^2 halves (8:24)

        def proj(w_sb, t8):
            """fp8 DR projection for token chunk t8 -> psum [P, D] f32."""
            ps = psum.tile([P, D], F32, tag="qkv_ps")
            for oc in range(2):
                for dp in range(4):
                    nc.tensor.matmul(
                        ps[:, oc * 512:(oc + 1) * 512],
                        x8T[:, 2 * dp:2 * dp + 2, t8 * P:(t8 + 1) * P],
                        w_sb[:, 2 * dp:2 * dp + 2, oc * 512:(oc + 1) * 512],
                        start=(dp == 0), stop=(dp == 3),
                        perf_mode=DRM,
                    )
            return ps

        def qk_process(ps, tbl_i, t8, is_q, dst_dram):
            """evac + rms + rope + aug store for a q/k chunk."""
            ta = qk_pool.tile([P, H, HD], BF, tag="ta")
            if not with_b1:
                # rms qk-norm divides out any per-token scale -> no LN inv
                nc.scalar.activation(
                    ta[:].rearrange("p h d -> p (h d)"), ps[:], AF.Identity)
            else:
                # bias breaks scale invariance: apply true LN inv, then bias
                co = 0 if is_q else D
                tb1 = qk_pool.tile([P, D], F32, tag="tab1")
                nc.scalar.activation(tb1[:], ps[:], AF.Identity,
                                     scale=inv16[:, t8:t8 + 1])
                nc.vector.tensor_tensor(
                    ta[:].rearrange("p h d -> p (h d)"), tb1[:],
                    b1r_sb[:, co:co + D], OP.add)
            sqs = qk_pool.tile([P, H, HD], BF, tag="sqs")
            if is_q:
                nc.gpsimd.tensor_tensor(sqs[:], ta[:], ta[:], OP.mult)
            else:
                nc.scalar.activation(sqs[:].rearrange("p h d -> p (h d)"),
                                     ta[:].rearrange("p h d -> p (h d)"),
                                     AF.Square)
            ss = st_pool.tile([P, H], F32, tag="rms_ss")
            nc.vector.reduce_sum(ss[:], sqs[:], axis=mybir.AxisListType.X)
            sd = st_pool.tile([P, H], F32, tag="rms_sd")
            nc.scalar.activation(sd[:], ss[:], AF.Sqrt, scale=1.0 / HD,
                                 bias=eps_sb[:])
            rs = st_pool.tile([P, H], BF, tag="rms_rs")
            with nc.allow_low_precision("bf16 rms scale ok at fp8 accuracy"):
                nc.vector.reciprocal(rs[:], sd[:])
            # rope: prods[p,h,j,d] = ta[p,h,d] * tb[p,j,d]
            tb = tqtk[:, tbl_i, None, :, :].to_broadcast((P, 1, 2, HD))
            prods = qk_pool.tile([P, H, 2, HD], BF, tag="prods")
            nc.vector.tensor_tensor(
                prods[:], ta[:, :, None, :].to_broadcast((P, H, 2, HD)),
                tb.to_broadcast((P, H, 2, HD)), OP.mult,
            )
            y = qk_pool.tile([P, H, HD], BF, tag="ropey")
            nc.vector.tensor_tensor(
                y[:].rearrange("p h (j d) -> p h j d", j=2),
                prods[:, :, :, 0:32], prods[:, :, :, 32:64], OP.add,
            )
            aug = aug_bufs[qk_process.flip][:]
            qk_process.flip ^= 1
            meng = nc.vector if (is_q or t8 == 7) else nc.gpsimd
            meng.tensor_tensor(
                aug[:, :, 0:64], y[:],
                rs[:, :, None].to_broadcast((P, H, HD)), OP.mult,
            )
            if is_q:
                cp = cq_psum.tile([P, H * 3], F32, tag="cqp")
                nc.tensor.matmul(
                    cp[:], coordsT_sb[:, t8 * P:(t8 + 1) * P], acat_sb[:],
                    start=True, stop=True,
                )
                nc.vector.tensor_copy(
                    out=aug[:, :, 64:67],
                    in_=cp[:].rearrange("p (h c) -> p h c", h=H))
            else:
                nc.vector.tensor_copy(
                    out=aug[:, :, 64:67],
                    in_=coords_sb[:, t8:t8 + 1, :].to_broadcast((P, H, 3)))
            return nc.sync.dma_start(
                dst_dram.rearrange("n h d -> n (h d)")[
                    t8 * P:(t8 + 1) * P, :],
                aug[:].rearrange("p h d -> p (h d)"))

        qk_process.flip = 0

        def ln1_stats(t8):
            # sum(x) via tiny ones-contractions on PE
            for s in range(8):
                nc.tensor.matmul(
                    sx_ps[:, t8:t8 + 1],
                    x8T[:, s, t8 * P:(t8 + 1) * P], ones8[:],
                    start=(s == 0), stop=(s == 7),
                )

        def ln1_sq_stats():
            # sum(x^2) from host-squared fp8 x; half-sized staging buffer
            with tc.tile_pool(name="xa", bufs=1) as xa:
                xsqT = xa.tile([P, 4, N], F8)
                for hf in range(2):
                    nc.sync.dma_start(
                        xsqT[:],
                        xsqT_d.rearrange("s p t -> p s t")[:, 4 * hf:4 * hf + 4])
                    for t8 in range(8):
                        co = 8 + 8 * hf + t8
                        for s in range(4):
                            nc.tensor.matmul(
                                sx_ps[:, co:co + 1],
                                xsqT[:, s, t8 * P:(t8 + 1) * P], ones8[:],
                                start=(s == 0), stop=(s == 3),
                            )
            sqa = st_pool.tile([P, 8], F32, tag="sqa")
            nc.vector.tensor_copy(out=sqa[:], in_=sx_ps[:, 8:16])
            nc.vector.tensor_tensor(ssq8[:], sqa[:], sx_ps[:, 16:24], OP.add)

        def do_v(t8):
            ps = proj(wv_sb, t8)
            if with_b1:
                tvb = qk_pool.tile([P, D], F32, tag="tvb")
                nc.scalar.activation(tvb[:], ps[:], AF.Identity,
                                     scale=inv32[:, t8:t8 + 1])
                nc.vector.tensor_tensor(
                    vtil[:, t8, :, 0:64],
                    tvb[:].rearrange("p (h d) -> p h d", h=H),
                    b1r_sb[:, 2 * D:3 * D].rearrange("p (h d) -> p h d", h=H),
                    OP.add)
            else:
                nc.scalar.activation(
                    vtil[:, t8, :, 0:64],
                    ps[:].rearrange("p (h d) -> p h d", h=H),
                    AF.Identity, scale=inv32[:, t8:t8 + 1],
                )

        def ln1_inv():
            nmv = st_pool.tile([P, 8], F32, tag="nmv")
            mu2 = st_pool.tile([P, 8], F32, tag="mu2v")
            varv = st_pool.tile([P, 8], F32, tag="varv")
            sdv = st_pool.tile([P, 8], F32, tag="sdv")
            nc.vector.tensor_scalar_mul(nmv[:], sx_ps[:, 0:8], 1.0 / D)
            nc.vector.tensor_tensor(mu2[:], nmv[:], nmv[:], OP.mult)
            nc.vector.scalar_tensor_tensor(
                varv[:], ssq8[:], 1.0 / D, mu2[:], op0=OP.mult,
                op1=OP.subtract)
            nc.scalar.activation(sdv[:], varv[:], AF.Sqrt, scale=1024.0,
                                 bias=epsk_sb[:])
            nc.vector.reciprocal(inv32[:], sdv[:])

        k_store = {}

        def k_chunk(t8):
            ps = proj(wk_sb, t8)
            k_store[t8] = qk_process(ps, 4 + t8, t8, False, kaug_d)

        # k0/k1 first (PE warm), then the LN1 stats + inv, then rest of k
        k_chunk(0)
        k_chunk(1)
        for t8 in range(8):
            ln1_stats(t8)
        ln1_sq_stats()
        ln1_inv()
        if with_b1:
            inv16 = small.tile([P, 8], F32)
            nc.vector.tensor_scalar_mul(inv16[:], inv32[:], 2.0)
        for t8 in range(2, 8):
            k_chunk(t8)
        for g in range(2):
            issue_k_group(g)
        for t4 in range(4):
            ps = proj(wq_sb, t4)
            qk_process(ps, t4, t4, True, qaug_d)
            do_v(2 * t4)
            do_v(2 * t4 + 1)
        for g in range(2):
            issue_q_group(g)

        phc_ctx.close()

        # ==================== phase D: attention per head ===================
        mlp_ctx = ExitStack()
        wo_pool = mlp_ctx.enter_context(tc.tile_pool(name="wo", bufs=1))
        wo_sb = wo_pool.tile([P, 32, D], F8)
        mlp_pers = mlp_ctx.enter_context(tc.tile_pool(name="mlp", bufs=1))
        x2t_pool = mlp_ctx.enter_context(tc.tile_pool(name="x2t", bufs=1))
        x2pre = x2t_pool.tile([P, 8, MY], BF)   # x1^T before LN2 scaling
        x2T8 = mlp_pers.tile([P, 8, MY], F8)
        owT_sb = x2t_pool.tile([P, 4, 2, D], F8)
        ow_ctx = ExitStack()
        ow_pool = ow_ctx.enter_context(tc.tile_pool(name="ow", bufs=1))
        ow_sb = ow_pool.tile([P, 8, D], F8)

        xr_r = xres_d.rearrange("(o p) f -> p o f", p=P)

        with tc.tile_pool(name="att", bufs=2) as att_pool, \
             tc.tile_pool(name="attsm", bufs=2) as attsm, \
             tc.tile_pool(name="apsum", bufs=3, space="PSUM") as apsum, \
             tc.tile_pool(name="opsum", bufs=2, space="PSUM") as opsum:
            for h in range(H):
                if h == 0:
                    issue_k_group(2)
                    issue_q_group(2)
                if h == 2:
                    ow_inst = nc.sync.dma_start(
                        ow_sb[:], ow8_d.rearrange("s p f -> p s f"))
                    tile.add_dep_helper(ow_inst.ins, tr_insts[1].ins,
                                        reason="ow after qT group 1")
                if h == 4:
                    issue_k_group(3)
                    issue_q_group(3)
                if h in (5, 7, 9, 11, 12, 13, 14, 15):
                    # stream wo (4MB) through mid-phase-D DMA slack
                    i = {5: 0, 7: 1, 9: 2, 11: 3, 12: 4, 13: 5, 14: 6, 15: 7}[h]
                    nc.sync.dma_start(
                        wo_sb[:, 4 * i:4 * i + 4],
                        wo8_d[4 * i:4 * i + 4].rearrange("s p f -> p s f"))
                kT = kts[h // 4][0:68, h % 4]
                qT = qts[h // 4][0:68, h % 4]
                expT = att_pool.tile([P, 8, MY], F8, tag="expT")
                for kc2 in range(4):
                    s_ps = apsum.tile([P, 2, MY], F32, tag="s_ps")
                    for j in range(2):
                        nc.tensor.matmul(
                            s_ps[:, j],
                            kT[:, (2 * kc2 + j) * P:(2 * kc2 + j + 1) * P], qT,
                            start=True, stop=True,
                        )
                    nc.scalar.activation(
                        expT[:, 2 * kc2:2 * kc2 + 2, :], s_ps[:],
                        AF.Exp, scale=0.125,
                    )
                o_ps = opsum.tile([66, MY], F32, tag="o_ps")
                for pp in range(4):
                    nc.tensor.matmul(
                        o_ps[:],
                        vtil[:, 2 * pp:2 * pp + 2, h, :],
                        expT[:, 2 * pp:2 * pp + 2, :],
                        start=(pp == 0), stop=(pp == 3),
                        perf_mode=DRM,
                    )
                rec = attsm.tile([1, MY], F32, tag="rec")
                nc.vector.reciprocal(rec[:], o_ps[64:65, :])
                bc = attsm.tile([64, MY], F32, tag="bc")
                nc.gpsimd.partition_broadcast(bc[:], rec[:])
                nc.vector.tensor_tensor(
                    oT_all[(h % 2) * 64:(h % 2) * 64 + 64, h // 2, :],
                    o_ps[0:64, :], bc[:], OP.mult,
                )

        # ================== phase E: out-proj + residual ====================
        nc.sync.dma_start(owT_sb[:], ow8T_d)
        with tc.tile_pool(name="epsum", bufs=2, space="PSUM") as epsum, \
             tc.tile_pool(name="eTpsum", bufs=2, space="PSUM") as eTpsum, \
             tc.tile_pool(name="xres", bufs=2) as xres_pool:
            xres_t = []
            for qc in range(2):
                xr = xres_pool.tile([P, D], F32, tag="xres")
                nc.sync.dma_start(xr[:], xr_r[:, qc])
                xres_t.append(xr)
            for qc in range(4):
                if qc < 2:
                    x_res_q = xres_t[qc]
                else:
                    x_res_q = xres_pool.tile([P, D], F32, tag="xres")
                    nc.sync.dma_start(x_res_q[:], xr_r[:, qc])
                xp = epsum.tile([P, D], F32, tag="xp")
                for oc in range(2):
                    for pp in range(4):
                        nc.tensor.matmul(
                            xp[:, oc * 512:(oc + 1) * 512],
                            oT_all[:, 2 * pp:2 * pp + 2, qc * P:(qc + 1) * P],
                            ow_sb[:, 2 * pp:2 * pp + 2, oc * 512:(oc + 1) * 512],
                            start=(pp == 0), stop=(pp == 3),
                            perf_mode=DRM,
                        )
                if with_bo1:
                    xb = st_pool.tile([P, D], F32, tag="xpb")
                    nc.vector.scalar_tensor_tensor(
                        xb[:], xp[:], 1.0 / SO, bo1r_sb[:], op0=OP.mult,
                        op1=OP.add)
                    nc.vector.tensor_tensor(
                        x1_sb[:, qc, :], x_res_q[:], xb[:], OP.add)
                else:
                    nc.vector.scalar_tensor_tensor(
                        x1_sb[:, qc, :], xp[:], 1.0 / SO, x_res_q[:],
                        op0=OP.mult, op1=OP.add)
            # transposed out-proj: x1^T = x^T + s*(Wo1 @ o)^T (fp8 x is fine:
            # feeds only the ls2-suppressed MLP branch)
            for dc in range(8):
                xpt = eTpsum.tile([P, MY], F32, tag="xpt")
                for i in range(4):
                    nc.tensor.matmul(
                        xpt[:],
                        owT_sb[:, i, :, dc * P:(dc + 1) * P],
                        oT_all[:, 2 * i:2 * i + 2, :],
                        start=(i == 0), stop=(i == 3),
                        perf_mode=DRM,
                    )
                eng = nc.vector if dc % 2 == 0 else nc.gpsimd
                if True:
                    nc.vector.scalar_tensor_tensor(
                        x2pre[:, dc, :], xpt[:], 1.0 / SO,
                        x8T[:, dc, 0:MY], op0=OP.mult, op1=OP.add)

        ow_ctx.close()

        # =================== phase F: LN2 scaling ==========================
        act_pool = mlp_ctx.enter_context(tc.tile_pool(name="actp", bufs=1))
        actT = act_pool.tile([P, 32, MY], F8)
        with tc.tile_pool(name="ln2", bufs=2) as ln2_pool:
            ssq2 = small.tile([P, 4], F32)
            sx2 = small.tile([P, 4], F32)
            inv2bf = small.tile([P, 4], BF)
            inv2row = small.tile([1, MY], BF)
            bc2 = small.tile([P, MY], BF)
            for qc in range(4):
                scr = ln2_pool.tile([P, D], BF, tag="ttr2_scr")
                nc.scalar.activation(scr[:], x1_sb[:, qc, :], AF.Square,
                                     accum_out=ssq2[:, qc:qc + 1])
                nc.vector.reduce_sum(
                    sx2[:, qc:qc + 1], x1_sb[:, qc, :],
                    axis=mybir.AxisListType.X)
                nm2 = st_pool.tile([P, 1], F32, tag="nm2")
                mu22 = st_pool.tile([P, 1], F32, tag="mu22")
                var2 = st_pool.tile([P, 1], F32, tag="var2")
                sd2 = st_pool.tile([P, 1], F32, tag="sd2")
                nc.vector.tensor_scalar_mul(nm2[:], sx2[:, qc:qc + 1], 1.0 / D)
                nc.vector.tensor_tensor(mu22[:], nm2[:], nm2[:], OP.mult)
                nc.vector.scalar_tensor_tensor(
                    var2[:], ssq2[:, qc:qc + 1], 1.0 / D, mu22[:],
                    op0=OP.mult, op1=OP.subtract)
                nc.scalar.activation(sd2[:], var2[:], AF.Sqrt, scale=1024.0,
                                     bias=epsk_sb[:])
                with nc.allow_low_precision("bf16 ln2 scale at fp8 accuracy"):
                    nc.vector.reciprocal(inv2bf[:, qc:qc + 1], sd2[:])
            # inv2 as a [1, MY] row (token t = qc*128 + p) via dram bounce
            nc.sync.dma_start(inv2_d, inv2bf[:])
            nc.sync.dma_start(
                inv2row[:].rearrange("o (q p) -> o q p", q=4),
                inv2_d.rearrange("p q -> q p")[None, :, :])
            nc.gpsimd.partition_broadcast(bc2[:], inv2row[:])
            for dc2 in range(4):
                eng = nc.vector if dc2 % 2 == 0 else nc.gpsimd
                eng.tensor_tensor(
                    x2T8[:, 2 * dc2:2 * dc2 + 2, :],
                    x2pre[:, 2 * dc2:2 * dc2 + 2, :],
                    bc2[:, None, :].to_broadcast((P, 2, MY)), OP.mult)

        # ============ phases G+H: MLP up / silu / down / out ================
        if True:

            with tc.tile_pool(name="w2", bufs=3) as w2_pool, \
                 tc.tile_pool(name="sil", bufs=2) as sil_pool, \
                 tc.tile_pool(name="gpsum", bufs=4, space="PSUM") as gpsum, \
                 tc.tile_pool(name="mpsum", bufs=1, space="PSUM") as mpsum:
                mps = [mpsum.tile([P, 512], F32, name=f"m_ps{qc}")
                       for qc in range(4)]

                def down_pair(pp, eh):
                    for qc in range(4):
                        nc.tensor.matmul(
                            mps[qc][:],
                            actT[:, 2 * pp:2 * pp + 2, qc * P:(qc + 1) * P],
                            wo_sb[:, 2 * pp:2 * pp + 2,
                                  eh * 512:(eh + 1) * 512],
                            start=(pp == 0), stop=(pp == 15),
                            perf_mode=DRM,
                        )

                for jj in range(32):
                    w2_t = w2_pool.tile([P, 8, 256], F8, tag="w2t")
                    nc.sync.dma_start(w2_t[:], w28_d[jj])
                    ups = []
                    for half in range(2):
                        up = gpsum.tile([P, MY], F32, tag="u_ps")
                        for dp in range(4):
                            nc.tensor.matmul(
                                up[:],
                                w2_t[:, 2 * dp:2 * dp + 2,
                                     half * P:(half + 1) * P],
                                x2T8[:, 2 * dp:2 * dp + 2, :],
                                start=(dp == 0), stop=(dp == 3),
                                perf_mode=DRM,
                            )
                        ups.append(up)
                    sil = sil_pool.tile([P, MY], F32, tag="sil")
                    nc.scalar.activation(sil[:], ups[0][:], AF.Silu,
                                         scale=1.0 / SQ)
                    nc.vector.scalar_tensor_tensor(
                        actT[:, jj, :], ups[1][:], 1.0 / SQ, sil[:],
                        op0=OP.mult, op1=OP.mult,
                    )
                    if jj % 2 == 1:
                        down_pair(jj // 2, 0)

                # second output half + final residual
                o_ts = [st_pool.tile([P, D], F32, name=f"outt{qc}")
                        for qc in range(4)]
                for eh in range(2):
                    if eh == 1:
                        for pp in range(16):
                            down_pair(pp, 1)
                    for qc in range(4):
                        sl = slice(eh * 512, (eh + 1) * 512)
                        if with_bo2:
                            ob = st_pool.tile([P, 512], F32, tag="outb")
                            nc.vector.scalar_tensor_tensor(
                                ob[:], mps[qc][:], 1.0 / SO, bo2r_sb[:, sl],
                                op0=OP.mult, op1=OP.add)
                            nc.vector.tensor_tensor(
                                o_ts[qc][:, sl], x1_sb[:, qc, sl], ob[:],
                                OP.add)
                        else:
                            nc.vector.scalar_tensor_tensor(
                                o_ts[qc][:, sl], mps[qc][:], 1.0 / SO,
                                x1_sb[:, qc, sl], op0=OP.mult, op1=OP.add)
                        nc.scalar.dma_start(
                            out_d[qc * P:(qc + 1) * P, sl], o_ts[qc][:, sl])

        mlp_ctx.close()

    nc.compile()
    return nc


# ---------------------------------------------------------------------------
# host side
# ---------------------------------------------------------------------------

_prog_cache = {}


def _get_program(flags):
    if flags not in _prog_cache:
        _prog_cache[flags] = build_program(*flags)
    return _prog_cache[flags]


def kernel(**inputs):
    x = _f32(inputs["x"])
    coords = _f32(inputs["coords"])
    rope_pos = np.asarray(inputs["rope_pos"])
    ln1_w, ln1_b = _f32(inputs["ln1_w"]), _f32(inputs["ln1_b"])
    qkv_w, qkv_b = _f32(inputs["qkv_w"]), _f32(inputs["qkv_b"])
    qnw, knw = _f32(inputs["q_norm_w"]), _f32(inputs["k_norm_w"])
    sq_w, sk_w = _f32(inputs["sq_w"]), _f32(inputs["sk_w"])
    out_w, out_b = _f32(inputs["out_w"]), _f32(inputs["out_b"])
    ls1 = _f32(inputs["ls1_g"])
    ln2_w, ln2_b = _f32(inputs["ln2_w"]), _f32(inputs["ln2_b"])
    w12_w, w12_b = _f32(inputs["w12_w"]), _f32(inputs["w12_b"])
    wo_w, wo_b = _f32(inputs["wo_w"]), _f32(inputs["wo_b"])
    ls2 = _f32(inputs["ls2_g"])

    # ---- weight folding ----
    # LN1 weight into qkv weights; mean-centering absorbs the LN mean.
    W1 = qkv_w * ln1_w[None, :]
    W1 = W1 - W1.mean(1, keepdims=True)
    b1 = qkv_w @ ln1_b + qkv_b
    perm = np.empty(HD, np.int64)
    perm[:32] = np.arange(32) * 2
    perm[32:] = np.arange(32) * 2 + 1
    permD = np.concatenate([h * HD + perm for h in range(H)])
    Wq = W1[:D][permD]
    Wk = W1[D:2 * D][permD]
    Wv = W1[2 * D:]
    b1p = np.concatenate([b1[:D][permD], b1[D:2 * D][permD], b1[2 * D:]])
    qnw_p, knw_p = qnw[perm], knw[perm]

    half = 32
    inv_freq = 1.0 / THETA ** (np.arange(half, dtype=np.float32) / half)
    freqs = rope_pos.astype(np.float32)[:, None] * inv_freq
    cos, sin = np.cos(freqs), np.sin(freqs)

    def rope_tbl(w):
        t = np.empty((N, 2, 64), np.float32)
        t[:, 0, :32] = cos * w[None, :32]
        t[:, 0, 32:] = -sin * w[None, 32:]
        t[:, 1, :32] = sin * w[None, :32]
        t[:, 1, 32:] = cos * w[None, 32:]
        return t

    tq = rope_tbl(qnw_p)
    tk = rope_tbl(knw_p)

    A_cat = np.concatenate(
        [SSP * SP_SCALE * sq_w[h * HD:(h + 1) * HD].T @ sk_w[h * HD:(h + 1) * HD]
         for h in range(H)], 1)  # (3, 48)

    Wo1 = out_w * ls1[:, None] * SO
    bo1 = ls1 * out_b
    W2 = w12_w * ln2_w[None, :]
    W2 = W2 - W2.mean(1, keepdims=True)
    b2 = w12_w @ ln2_b + w12_b
    Wo2 = wo_w * ls2[:, None] * SO
    bo2 = ls2 * wo_b

    with_b1 = bool(np.any(b1p != 0))
    with_bo1 = bool(np.any(bo1 != 0))
    with_bo2 = bool(np.any(bo2 != 0))
    assert not np.any(b2 != 0), "nonzero w12 bias not supported by this kernel"
    flags = (with_b1, with_bo1, with_bo2)
    nc = _get_program(flags)

    # fp8 weight packs; contraction d-blocks on dim 0 of each 128-block
    def pack_qkv(W):  # (D_out rows, D in) -> (8, 128, D_out) d-block major
        WT = np.ascontiguousarray(W.T)              # (D, D_out)
        return _q8(WT.reshape(8, P, -1))

    wqkv = np.stack([pack_qkv(16.0 * Wq), pack_qkv(16.0 * Wk),
                     pack_qkv(SQ * Wv)])
    ow8 = _q8(np.ascontiguousarray(Wo1.T).reshape(8, P, D))
    ow8T = _q8(np.ascontiguousarray(
        Wo1.T.reshape(4, 2, P, D).transpose(2, 0, 1, 3)))
    # w28: [32 jj, 128 p, 8 dblock, 256] partition-contiguous,
    # cols [a(128) | b(128)]
    W2T = np.ascontiguousarray((SQ * W2).T)          # (D, 8192)
    w28 = np.empty((32, 8, P, 256), np.float32)
    W2Tr = W2T.reshape(8, P, 2 * HID)
    for jj in range(32):
        w28[jj, :, :, 0:128] = W2Tr[:, :, jj * P:(jj + 1) * P]
        w28[jj, :, :, 128:256] = W2Tr[:, :, HID + jj * P:HID + (jj + 1) * P]
    w28 = _q8(np.transpose(w28, (0, 2, 1, 3)))
    wo8 = _q8(np.ascontiguousarray(Wo2.T).reshape(32, P, D))

    shared = {
        "a_cat": _bf(A_cat),
        "wqkv": wqkv, "ow8": ow8, "ow8T": ow8T, "w28": w28, "wo8": wo8,
    }
    if with_b1:
        shared["b1rep"] = _f32(np.broadcast_to(b1p[None, :], (P, 3 * D)))
    if with_bo1:
        shared["bo1rep"] = _f32(np.broadcast_to(bo1[None, :], (P, D)))
    if with_bo2:
        shared["bo2rep"] = _f32(np.broadcast_to(bo2[None, :], (P, D)))

    in_maps = []
    for c in range(NCORES):
        b, r = c // 2, c % 2
        rows = np.concatenate([np.arange(r * MY, (r + 1) * MY),
                               np.arange(0, r * MY),
                               np.arange((r + 1) * MY, N)])
        xb = x[b][rows]
        x8 = _q8(xb)
        x8f = x8.astype(np.float32)
        m = dict(shared)
        m["x8T"] = np.ascontiguousarray(x8.T.reshape(8, P, N))
        m["xsqT"] = _q8((x8f * x8f).T.reshape(8, P, N))
        m["xres"] = _f32(xb[:MY])
        m["coords_tm"] = _bf(coords[b][rows])
        m["coordsT"] = _bf(coords[b][rows[:MY]].T)
        m["tq"] = _bf(tq[r * MY:(r + 1) * MY])
        m["tk"] = _bf(tk[rows])
        in_maps.append(m)

    res = run_bass_kernel_spmd(nc, in_maps, core_ids=list(range(NCORES)),
                               trace=bool(int(os.environ.get("K_TRACE", "0"))))
    out = np.empty((B, N, D), np.float32)
    for c in range(NCORES):
        b, r = c // 2, c % 2
        out[b, r * MY:(r + 1) * MY] = res.results[c]["out"]
    kernel.last_result = res
    return out
